# revision 18
# baseline (speedup 1.0000x reference)
"""AraBERT-CharBiLSTM-CRF forward loss on 8 TRN2 NeuronCores.

Data-parallel over batch: 8 examples/core. Per core:
  proj+LN+ReLU (feature-major via PE transposes) -> x0T
  gx = x0T @ W_ih (precomputed, streamed via DRAM)  [bf16 matmuls]
  2-layer BiLSTM scan, batch-on-partition layout, identity-matmul gx inject
  FC emissions (token-major), CRF score via one-hot matmuls,
  CRF partition via exp-domain matvec scan with periodic renorm.
Output: per-core partial loss, summed on host.
"""
import os
import numpy as np
import ml_dtypes

import concourse.bass as bass
import concourse.bacc as bacc_mod
import concourse.mybir as mybir
import concourse.tile as tile
from concourse.bass_utils import run_bass_kernel_spmd

F32 = mybir.dt.float32
BF16 = mybir.dt.bfloat16
AF = mybir.ActivationFunctionType
ALU = mybir.AluOpType
AX = mybir.AxisListType

NCORES = 8
BL = 8          # examples per core
T = 512
V = BL * T      # 4096 tokens per core
D = 768
PF = 384        # projected dim
CE = 100
IN0 = PF + CE   # 484
HD = 256
G4 = 4 * HD     # 1024
K = 15
NCH = 32        # 4096/128 v-chunks
EPS = 1e-5
RENORM = 8

bf = ml_dtypes.bfloat16


def build_nc():
    nc = bacc_mod.Bacc(None, target_bir_lowering=False)
    P = {}

    def par(name, shape, dt, out=False):
        P[name] = nc.declare_dram_parameter(name, list(shape), dt, isOutput=out)
        return P[name]

    # per-core shards
    par("emb", (V, D), F32)
    par("tags", (BL, T), F32)
    par("chars", (BL, T), F32)
    # weights (bf16, host-prepped)
    par("wp", (128, 7 * PF), BF16)          # 6 ktiles + bias row (part 0 of blk 6)
    par("chtab", (CE, CE), BF16)
    for d in range(2):
        par(f"wih0_{d}", (128, 4 * G4), BF16)   # ktiles 0-2 proj, 3 = char rows
        par(f"whh0_{d}", (128, 2 * G4), BF16)
        par(f"wih1_{d}", (128, 4 * G4), BF16)
        par(f"whh1_{d}", (128, 2 * G4), BF16)
        par(f"b0_{d}", (1, G4), BF16)
        par(f"b1_{d}", (1, G4), BF16)
    par("wfc", (128, 4 * K), BF16)
    par("bfc", (1, K), BF16)
    # consts f32
    par("gamma", (128, 3), F32)
    par("beta", (128, 3), F32)
    par("iota100", (128, CE), F32)
    par("iota15", (128, K), F32)
    par("notlast", (128, NCH), F32)
    par("id128f", (128, 128), F32)
    par("transf", (K, K), F32)
    par("eexp", (K, K), F32)
    par("estart", (K, 1), F32)
    par("eend", (K, 1), F32)
    par("startrep", (BL, K), F32)
    par("endrep", (BL, K), F32)
    par("ones128f", (128, 1), F32)
    par("ones15f", (K, 1), F32)
    par("ones1x15f", (1, K), F32)
    # consts bf16
    par("id128b", (128, 128), BF16)
    par("id8b", (BL, BL), BF16)
    par("id8f", (BL, BL), F32)
    par("out", (1, 1), F32, out=True)

    # DRAM scratch
    gx = {}
    for l in range(2):
        for d in range(2):
            gx[(l, d)] = nc.dram_tensor(f"gx{l}{d}", [BL, T, G4], BF16)
    # windowed layout: [dir, t//128, p, k*1024 + b*128 + t%128]
    x1t = nc.dram_tensor("x1t", [2, 4, 128, 2 * BL * 128], BF16)
    x2t = nc.dram_tensor("x2t", [2, 4, 128, 2 * BL * 128], BF16)

    with tile.TileContext(nc) as tc:
        build_body(nc, tc, P, gx, x1t, x2t)
    nc.finalize()
    return nc


def build_body(nc, tc, P, gx, x1t, x2t):
    dma = nc.sync.dma_start
    from contextlib import ExitStack
    ctx = ExitStack()

    wpool = ctx.enter_context(tc.tile_pool(name="weights", bufs=1))

    def load(name, dt=None):
        p = P[name]
        t = wpool.tile(list(p.shape), p.dtype if dt is None else dt,
                       name=f"w_{name}", tag=f"w_{name}")
        dma(out=t[:], in_=p.ap())
        return t

    wp_sb = load("wp")
    chtab_sb = load("chtab")
    wih0 = [load(f"wih0_{d}") for d in range(2)]
    whh0 = [load(f"whh0_{d}") for d in range(2)]
    wih1 = [load(f"wih1_{d}") for d in range(2)]
    whh1 = [load(f"whh1_{d}") for d in range(2)]
    wfc_sb = load("wfc")
    bfc_sb = load("bfc")
    b0 = [load(f"b0_{d}") for d in range(2)]
    b1 = [load(f"b1_{d}") for d in range(2)]
    gamma_sb = load("gamma")
    beta_sb = load("beta")
    iota100_sb = load("iota100")
    iota15_sb = load("iota15")
    notlast_sb = load("notlast")
    id128f_sb = load("id128f")
    id128b_sb = load("id128b")
    id8b_sb = load("id8b")
    id8f_sb = load("id8f")
    transf_sb = load("transf")
    eexp_sb = load("eexp")
    estart_sb = load("estart")
    eend_sb = load("eend")
    startrep_sb = load("startrep")
    endrep_sb = load("endrep")
    ones128f_sb = load("ones128f")
    ones15f_sb = load("ones15f")
    ones1x15f_sb = load("ones1x15f")

    # strided loads: tags/chars as (128, 32) v-major
    tags_sb = wpool.tile([128, NCH], F32)
    chars_sb = wpool.tile([128, NCH], F32)
    tags_flat = P["tags"].ap().rearrange("b t -> (b t)")
    chars_flat = P["chars"].ap().rearrange("b t -> (b t)")
    dma(out=tags_sb[:], in_=tags_flat.rearrange("(c p) -> p c", p=128))
    dma(out=chars_sb[:], in_=chars_flat.rearrange("(c p) -> p c", p=128))
    tags_t0 = wpool.tile([BL, 1], F32)
    tags_tl = wpool.tile([BL, 1], F32)
    dma(out=tags_t0[:], in_=P["tags"].ap()[:, 0:1])
    dma(out=tags_tl[:], in_=P["tags"].ap()[:, T - 1:T])

    ones1x128b = wpool.tile([1, 128], BF16)
    nc.vector.memset(ones1x128b[:], 1.0)
    eps_sb = wpool.tile([128, 1], F32)
    nc.vector.memset(eps_sb[:], EPS)

    emb_ap = P["emb"].ap()

    # PE warmup: absorb const-DMA waits into multi-wait-capable matmuls so
    # later transpose instructions (single-wait TR struct) never carry them.
    with tc.tile_pool(name="warm_ps", bufs=1, space="PSUM") as wps:
        wm1 = wps.tile([128, 128], F32, name="wm1")
        nc.tensor.matmul(wm1[:], id128f_sb[:], id128f_sb[:], start=True, stop=True)
        wm2 = wps.tile([BL, BL], F32, name="wm2")
        nc.tensor.matmul(wm2[:], id8b_sb[:], id8b_sb[:], start=True, stop=True)
        wm3 = wps.tile([BL, BL], F32, name="wm3")
        nc.tensor.matmul(wm3[:], id8f_sb[:], id8f_sb[:], start=True, stop=True)

    # ---------------- Phase B: proj + LN + char + gx L0 ----------------
    with tc.tile_pool(name="phb_sb", bufs=3) as sbp, \
         tc.tile_pool(name="phb_x0", bufs=1) as x0pool, \
         tc.tile_pool(name="phb_ps", bufs=2, space="PSUM") as psp, \
         tc.tile_pool(name="phb_ps2", bufs=1, space="PSUM") as psp2, \
         tc.tile_pool(name="phb_sb2", bufs=3) as sbp2:
        x0p = [x0pool.tile([128, V], BF16, name=f'x0p{i}') for i in range(3)]
        x0c = x0pool.tile([CE, V], BF16)
        for c in range(NCH):
            v0 = c * 128
            embc = sbp.tile([128, D], F32, tag="embc")
            dma(out=embc[:], in_=emb_ap[v0:v0 + 128, :])
            embT = sbp.tile([128, D], BF16, tag="embT")
            for k in range(6):
                tp = psp.tile([128, 128], F32, tag="tp")
                nc.tensor.transpose(tp[:], embc[:, k * 128:(k + 1) * 128], id128f_sb[:])
                nc.vector.tensor_copy(embT[:, k * 128:(k + 1) * 128], tp[:])
            pj = psp2.tile([128, PF], F32, tag="pj")
            for k in range(6):
                nc.tensor.matmul(pj[:], embT[:, k * 128:(k + 1) * 128],
                                 wp_sb[:, k * PF:(k + 1) * PF],
                                 start=(k == 0), stop=False)
            nc.tensor.matmul(pj[:], ones1x128b[:], wp_sb[0:1, 6 * PF:7 * PF],
                             start=False, stop=True)
            # LN: var via Square+accum, rstd = exp(-0.5*ln(ss/384+eps))
            junk = sbp.tile([128, PF], BF16, tag="junk")
            ss = sbp.tile([128, 1], F32, tag="ss")
            nc.scalar.activation(junk[:], pj[:], AF.Square, accum_out=ss[:])
            lnv = sbp.tile([128, 1], F32, tag="lnv")
            nc.scalar.activation(lnv[:], ss[:], AF.Ln, bias=eps_sb[:], scale=1.0 / PF)
            rstd = sbp.tile([128, 1], F32, tag="rstd")
            nc.scalar.activation(rstd[:], lnv[:], AF.Exp, scale=-0.5)
            xn = sbp.tile([128, PF], F32, tag="xn")
            nc.vector.tensor_scalar_mul(xn[:], pj[:], rstd[:])
            for k in range(3):
                tp = psp.tile([128, 128], F32, tag="tp")
                nc.tensor.transpose(tp[:], xn[:, k * 128:(k + 1) * 128], id128f_sb[:])
                gb = sbp.tile([128, 128], F32, tag="gb")
                nc.vector.tensor_scalar(gb[:], tp[:], gamma_sb[:, k:k + 1],
                                        beta_sb[:, k:k + 1], op0=ALU.mult, op1=ALU.add)
                nc.vector.tensor_scalar_max(x0p[k][:, v0:v0 + 128], gb[:], 0.0)
            # char one-hot -> transpose -> char_embT
            ohc = sbp.tile([128, CE], F32, tag="ohc")
            nc.vector.tensor_scalar(ohc[:], iota100_sb[:], chars_sb[:, c:c + 1], None,
                                    op0=ALU.is_equal)
            tpc = psp.tile([128, 128], F32, tag="tpc")
            nc.tensor.transpose(tpc[:CE, :], ohc[:], id128f_sb[:])
            cemT = sbp.tile([CE, 128], BF16, tag="cemT")
            nc.vector.tensor_copy(cemT[:], tpc[:CE, :])
            cet = psp2.tile([CE, 128], F32, tag="cet")
            nc.tensor.matmul(cet[:], chtab_sb[:], cemT[:], start=True, stop=True)
            nc.vector.tensor_copy(x0c[:, v0:v0 + 128], cet[:])
        # gx L0
        for c in range(NCH):
            v0 = c * 128
            b = c // 4
            t0 = (c % 4) * 128
            for d in range(2):
                gout = sbp2.tile([128, G4], BF16, tag="gout")
                for h in range(2):
                    gp = psp2.tile([128, 512], F32, tag="gp")
                    for k in range(3):
                        nc.tensor.matmul(gp[:], x0p[k][:, v0:v0 + 128],
                                         wih0[d][:, k * G4 + h * 512:k * G4 + (h + 1) * 512],
                                         start=(k == 0), stop=False)
                    nc.tensor.matmul(gp[:], x0c[:, v0:v0 + 128],
                                     wih0[d][0:CE, 3 * G4 + h * 512:3 * G4 + (h + 1) * 512],
                                     start=False, stop=False)
                    nc.tensor.matmul(gp[:], ones1x128b[:],
                                     b0[d][:, h * 512:(h + 1) * 512],
                                     start=False, stop=True)
                    nc.vector.tensor_copy(gout[:, h * 512:(h + 1) * 512], gp[:])
                dma(out=gx[(0, d)].ap()[b, t0:t0 + 128, :], in_=gout[:])

    # ---------------- scans ----------------
    scan_layer(nc, tc, 0, [whh0[0], whh0[1]], gx, x1t, dma, id8b_sb, id8f_sb)
    gx1_phase(nc, tc, x1t, wih1, gx, ones1x128b, b1, dma)
    scan_layer(nc, tc, 1, [whh1[0], whh1[1]], gx, x2t, dma, id8b_sb, id8f_sb)

    # ---------------- FC + CRF ----------------
    with tc.tile_pool(name="fc_sb", bufs=3) as sbf, \
         tc.tile_pool(name="fc_ps", bufs=1, space="PSUM") as psf, \
         tc.tile_pool(name="crf_sb", bufs=1) as sbc, \
         tc.tile_pool(name="crf_ps", bufs=1, space="PSUM") as psc, \
         tc.tile_pool(name="crf_ps2", bufs=1, space="PSUM") as psc2:
        emT = sbc.tile([K, V], F32)
        ohall = sbc.tile([128, NCH * K], BF16)
        ohm = sbc.tile([128, NCH * K], BF16)
        emsc = sbc.tile([128, 1], F32)
        nc.vector.memset(emsc[:], 0.0)
        for c in range(NCH):
            v0 = c * 128
            b = c // 4
            t0 = (c % 4) * 128
            w = c % 4
            h1 = sbf.tile([128, 4 * 128], BF16, tag="h1")
            for k in range(4):
                dd, kin = k // 2, k % 2
                dma(out=h1[:, k * 128:(k + 1) * 128],
                    in_=x2t.ap()[dd, w, :, kin * 1024 + b * 128:kin * 1024 + b * 128 + 128])
            emp = psf.tile([128, K], F32, tag="emp")
            for k in range(4):
                nc.tensor.matmul(emp[:], h1[:, k * 128:(k + 1) * 128],
                                 wfc_sb[:, k * K:(k + 1) * K],
                                 start=(k == 0), stop=False)
            nc.tensor.matmul(emp[:], ones1x128b[:], bfc_sb[:],
                             start=False, stop=True)
            em = sbf.tile([128, K], F32, tag="em")
            nc.vector.tensor_copy(em[:], emp[:])
            etp = psf.tile([K, 128], F32, tag="etp")
            nc.tensor.transpose(etp[:], em[:], id128f_sb[:])
            nc.vector.tensor_copy(emT[:, v0:v0 + 128], etp[:])
            # score pieces
            ohf = sbf.tile([128, K], F32, tag="ohf")
            nc.vector.tensor_scalar(ohf[:], iota15_sb[:], tags_sb[:, c:c + 1], None,
                                    op0=ALU.is_equal)
            prod = sbf.tile([128, K], F32, tag="prod")
            nc.vector.tensor_tensor(prod[:], em[:], ohf[:], op=ALU.mult)
            red = sbf.tile([128, 1], F32, tag="red")
            nc.vector.tensor_reduce(red[:], prod[:], axis=AX.X, op=ALU.add)
            nc.vector.tensor_tensor(emsc[:], emsc[:], red[:], op=ALU.add)
            nc.vector.tensor_copy(ohall[:, c * K:(c + 1) * K], ohf[:])
            nc.vector.tensor_scalar_mul(ohm[:, c * K:(c + 1) * K],
                                        ohall[:, c * K:(c + 1) * K],
                                        notlast_sb[:, c:c + 1])
        # shifted one-hots (v+1) via DMA
        ohnext = sbc.tile([128, NCH * K], BF16)
        nc.vector.memset(ohnext[:], 0.0)
        oh3 = ohall[:].rearrange("p (c k) -> p c k", k=K)
        on3 = ohnext[:].rearrange("p (c k) -> p c k", k=K)
        dma(out=on3[0:127, :, :], in_=oh3[1:128, :, :])
        dma(out=on3[127:128, 0:NCH - 1, :], in_=oh3[0:1, 1:NCH, :])
        cps = psc.tile([K, K], F32)
        for c in range(NCH):
            nc.tensor.matmul(cps[:], ohm[:, c * K:(c + 1) * K],
                             ohnext[:, c * K:(c + 1) * K],
                             start=(c == 0), stop=(c == NCH - 1))
        trprod = sbc.tile([K, K], F32)
        nc.vector.tensor_tensor(trprod[:], cps[:], transf_sb[:], op=ALU.mult)
        trred = sbc.tile([K, 1], F32)
        nc.vector.tensor_reduce(trred[:], trprod[:], axis=AX.X, op=ALU.add)
        # start/end
        oh0 = sbc.tile([BL, K], F32)
        nc.vector.tensor_scalar(oh0[:], iota15_sb[:BL, :], tags_t0[:], None, op0=ALU.is_equal)
        ohl = sbc.tile([BL, K], F32)
        nc.vector.tensor_scalar(ohl[:], iota15_sb[:BL, :], tags_tl[:], None, op0=ALU.is_equal)
        sst = sbc.tile([BL, 1], F32)
        pr0 = sbc.tile([BL, K], F32)
        nc.vector.tensor_tensor(pr0[:], oh0[:], startrep_sb[:], op=ALU.mult)
        nc.vector.tensor_reduce(sst[:], pr0[:], axis=AX.X, op=ALU.add)
        sen = sbc.tile([BL, 1], F32)
        prl = sbc.tile([BL, K], F32)
        nc.vector.tensor_tensor(prl[:], ohl[:], endrep_sb[:], op=ALU.mult)
        nc.vector.tensor_reduce(sen[:], prl[:], axis=AX.X, op=ALU.add)
        # total score
        scps = psc.tile([1, 1], F32)
        nc.tensor.matmul(scps[:], emsc[:], ones128f_sb[:], start=True, stop=False)
        nc.tensor.matmul(scps[:], trred[:], ones15f_sb[:], start=False, stop=False)
        nc.tensor.matmul(scps[:], sst[:], ones128f_sb[:BL, :], start=False, stop=False)
        nc.tensor.matmul(scps[:], sen[:], ones128f_sb[:BL, :], start=False, stop=True)

        # ---- z-scan ----
        w_sb = sbc.tile([K, V], F32)
        nc.scalar.activation(w_sb[:], emT[:], AF.Exp)
        z = sbc.tile([K, BL], F32)
        wv = w_sb[:].rearrange("k (b t) -> k t b", t=T)
        nc.vector.tensor_scalar_mul(z[:], wv[:, 0, :], estart_sb[:])
        slog = sbc.tile([1, 512], F32)
        nc.vector.memset(slog[:], 1.0)
        ridx = 0
        for t in range(1, T):
            zp = psc2.tile([K, BL], F32, tag="zp")
            nc.tensor.matmul(zp[:], eexp_sb[:], z[:], start=True, stop=True)
            nc.vector.tensor_tensor(z[:], zp[:], wv[:, t, :], op=ALU.mult)
            if t % RENORM == 0:
                sps = psc2.tile([1, BL], F32, tag="sps")
                nc.tensor.matmul(sps[:], ones15f_sb[:], z[:], start=True, stop=True)
                nc.vector.tensor_copy(slog[:, ridx * BL:(ridx + 1) * BL], sps[:])
                rr = sbc.tile([1, BL], F32, tag="rr")
                nc.vector.reciprocal(rr[:], sps[:])
                bps = psc2.tile([K, BL], F32, tag="bps")
                nc.tensor.matmul(bps[:], ones1x15f_sb[:], rr[:], start=True, stop=True)
                nc.vector.tensor_tensor(z[:], z[:], bps[:], op=ALU.mult)
                ridx += 1
        zf = sbc.tile([K, BL], F32)
        nc.vector.tensor_scalar_mul(zf[:], z[:], eend_sb[:])
        sfin = psc2.tile([1, BL], F32, tag="sps")
        nc.tensor.matmul(sfin[:], ones15f_sb[:], zf[:], start=True, stop=True)
        lnf = sbc.tile([1, BL], F32)
        nc.scalar.activation(lnf[:], sfin[:], AF.Ln)
        lslog = sbc.tile([1, 512], F32)
        nc.scalar.activation(lslog[:], slog[:], AF.Ln)
        lsred = sbc.tile([1, BL], F32)
        ls3 = lslog[:].rearrange("p (r b) -> p b r", b=BL)
        nc.vector.tensor_reduce(lsred[:], ls3, axis=AX.X, op=ALU.add)
        logzb = sbc.tile([1, BL], F32)
        nc.vector.tensor_tensor(logzb[:], lnf[:], lsred[:], op=ALU.add)
        logzs = sbc.tile([1, 1], F32)
        nc.vector.tensor_reduce(logzs[:], logzb[:], axis=AX.X, op=ALU.add)
        outsb = sbc.tile([1, 1], F32)
        nc.vector.scalar_tensor_tensor(outsb[:], scps[:], -1.0, logzs[:],
                                       op0=ALU.mult, op1=ALU.add)
        dma(out=P["out"].ap(), in_=outsb[:])


def scan_layer(nc, tc, layer, whh, gx, xout_dram, dma, id8b_sb, id8f_sb):
    """One BiLSTM layer: fwd+bwd scans interleaved, writing transposed
    (2h-scaled) hidden states to xout_dram (2HD, BL, T) bf16."""
    with tc.tile_pool(name=f"sc{layer}_st", bufs=1) as stp, \
         tc.tile_pool(name=f"sc{layer}_gw", bufs=2) as gwp, \
         tc.tile_pool(name=f"sc{layer}_asm", bufs=2) as asmp, \
         tc.tile_pool(name=f"sc{layer}_pst", bufs=2, space="PSUM") as pst, \
         tc.tile_pool(name=f"sc{layer}_ps", bufs=1, space="PSUM") as psg:
        st = []
        for d in range(2):
            s = dict(
                ch=stp.tile([BL, HD], F32, name=f"ch{layer}{d}"),
                tg=stp.tile([BL, G4], F32, name=f"tg{layer}{d}"),
                A=stp.tile([BL, HD], F32, name=f"A{layer}{d}"),
                Bv=stp.tile([BL, HD], F32, name=f"Bv{layer}{d}"),
                tcc=stp.tile([BL, HD], F32, name=f"tcc{layer}{d}"),
                hh=stp.tile([BL, HD], F32, name=f"hh{layer}{d}"),
                h0=stp.tile([128, 2 * BL], BF16, name=f"h0{layer}{d}"),
            )
            nc.vector.memset(s["ch"][:], 0.0)
            nc.vector.memset(s["h0"][:], 0.0)
            st.append(s)
        gwin = [None, None]
        asm = [None, None]
        for t_idx in range(T):
            for d in range(2):
                t = t_idx if d == 0 else T - 1 - t_idx
                s = st[d]
                if t_idx % 4 == 0:
                    w4 = t // 4
                    gwin[d] = gwp.tile([BL, 4 * G4], BF16, name=f"gw{d}", tag=f"gw{d}")
                    gw3 = gwin[d][:].rearrange("b (t g) -> b t g", g=G4)
                    dma(out=gw3[:, :, :], in_=gx[(layer, d)].ap()[:, w4 * 4:(w4 + 1) * 4, :])
                if t_idx % 128 == 0:
                    asm[d] = asmp.tile([128, 2 * BL * 128], BF16, name=f"asm{d}", tag=f"asm{d}")
                toff = t % 4
                tcol = t % 128
                gslice = gwin[d][:].rearrange("b (t g) -> b t g", g=G4)[:, toff, :]
                if t_idx == 0:
                    lhs = [s["h0"][:, 0:BL], s["h0"][:, BL:2 * BL]]
                else:
                    a3 = asm[d][:].rearrange("p (k b t) -> p k b t", k=2, b=BL)
                    pcol = (t - 1) if d == 0 else (t + 1)
                    if (d == 0 and t % 128 == 0) or (d == 1 and t % 128 == 127):
                        lhs = [s["hT"][0], s["hT"][1]]
                    else:
                        lhs = [a3[:, 0, :, pcol % 128], a3[:, 1, :, pcol % 128]]
                gp = psg.tile([BL, G4], F32, tag=f"g{d}")
                for h in range(2):
                    nc.tensor.matmul(gp[:, h * 512:(h + 1) * 512], lhs[0],
                                     whh[d][:, h * 512:(h + 1) * 512],
                                     start=True, stop=False)
                    nc.tensor.matmul(gp[:, h * 512:(h + 1) * 512], lhs[1],
                                     whh[d][:, G4 + h * 512:G4 + (h + 1) * 512],
                                     start=False, stop=False)
                    nc.tensor.matmul(gp[:, h * 512:(h + 1) * 512], id8b_sb[:],
                                     gslice[:, h * 512:(h + 1) * 512],
                                     start=False, stop=True)
                nc.scalar.activation(s["tg"][:], gp[:], AF.Tanh)
                tgv = s["tg"]
                nc.vector.scalar_tensor_tensor(s["A"][:], tgv[:, HD:2 * HD], 1.0,
                                               s["ch"][:], op0=ALU.add, op1=ALU.mult)
                nc.vector.scalar_tensor_tensor(s["Bv"][:], tgv[:, 0:HD], 1.0,
                                               tgv[:, 3 * HD:4 * HD], op0=ALU.add, op1=ALU.mult)
                nc.vector.scalar_tensor_tensor(s["ch"][:], s["A"][:], 0.5,
                                               s["Bv"][:], op0=ALU.mult, op1=ALU.add)
                nc.scalar.activation(s["tcc"][:], s["ch"][:], AF.Tanh, scale=0.5)
                nc.vector.scalar_tensor_tensor(s["hh"][:], tgv[:, 2 * HD:3 * HD], 1.0,
                                               s["tcc"][:], op0=ALU.add, op1=ALU.mult)
                a3 = asm[d][:].rearrange("p (k b t) -> p k b t", k=2, b=BL)
                hT = []
                for kk in range(2):
                    trp = pst.tile([128, BL], F32, tag=f"tr{d}")
                    nc.tensor.transpose(trp[:], s["hh"][:, kk * 128:(kk + 1) * 128],
                                        id8f_sb[:])
                    nc.vector.tensor_copy(a3[:, kk, :, tcol], trp[:])
                    hT.append(a3[:, kk, :, tcol])
                s["hT"] = hT
                if t_idx % 128 == 127:
                    dma(out=xout_dram.ap()[d, t // 128], in_=asm[d][:])


def gx1_phase(nc, tc, x1t, wih1, gx, ones1x128b, b1, dma):
    with tc.tile_pool(name="g1_sb", bufs=3) as sbp, \
         tc.tile_pool(name="g1_ps", bufs=2, space="PSUM") as psp:
        for c in range(NCH):
            b = c // 4
            t0 = (c % 4) * 128
            w = c % 4
            xt = sbp.tile([128, 4 * 128], BF16, tag="xt")
            for k in range(4):
                dd, kin = k // 2, k % 2
                dma(out=xt[:, k * 128:(k + 1) * 128],
                    in_=x1t.ap()[dd, w, :, kin * 1024 + b * 128:kin * 1024 + b * 128 + 128])
            for d in range(2):
                gout = sbp.tile([128, G4], BF16, tag="gout")
                for h in range(2):
                    gp = psp.tile([128, 512], F32, tag="gp")
                    for k in range(4):
                        nc.tensor.matmul(gp[:], xt[:, k * 128:(k + 1) * 128],
                                         wih1[d][:, k * G4 + h * 512:k * G4 + (h + 1) * 512],
                                         start=(k == 0), stop=False)
                    nc.tensor.matmul(gp[:], ones1x128b[:],
                                     b1[d][:, h * 512:(h + 1) * 512],
                                     start=False, stop=True)
                    nc.vector.tensor_copy(gout[:, h * 512:(h + 1) * 512], gp[:])
                dma(out=gx[(1, d)].ap()[b, t0:t0 + 128, :], in_=gout[:])


# ---------------------------------------------------------------- host side

def _pack_k(a, kt):
    """(rows, cols) -> (128, kt*cols): ktile k at [:, k*cols:(k+1)*cols]."""
    rows, cols = a.shape
    pad = np.zeros((kt * 128, cols), a.dtype)
    pad[:rows] = a
    return pad.reshape(kt, 128, cols).transpose(1, 0, 2).reshape(128, kt * cols).copy()


def _prep(inputs):
    """Host-side weight preprocessing -> dict of replicated arrays."""
    f32 = np.float32
    pW = np.asarray(inputs["proj_W"], f32)
    pb = np.asarray(inputs["proj_b"], f32)
    Pc = np.eye(PF, dtype=f32) - 1.0 / PF
    Wc = Pc @ pW
    bc = Pc @ pb
    wp = _pack_k(np.concatenate([Wc.T, bc[None, :]], 0), 7)

    perm = np.concatenate([np.arange(0, HD), np.arange(HD, 2 * HD),
                           np.arange(3 * HD, 4 * HD), np.arange(2 * HD, 3 * HD)])
    lp = inputs["lstm_params"]

    def mk(layer, d, scale_ih):
        p = lp[layer]["fwd" if d == 0 else "bwd"]
        wih = np.asarray(p["w_ih"], f32)[perm] * scale_ih
        whh = np.asarray(p["w_hh"], f32)[perm] * 0.5
        b = np.asarray(p["b"], f32)[perm]
        return (_pack_k(wih.T.copy(), 4), _pack_k(whh.T.copy(), 2),
                b[None, :].copy())

    out = {"wp": wp.astype(bf),
           "chtab": np.asarray(inputs["char_table"], f32).astype(bf)}
    for d in range(2):
        a, b_, bias = mk(0, d, 1.0)
        out[f"wih0_{d}"] = a.astype(bf)
        out[f"whh0_{d}"] = b_.astype(bf)
        out[f"b0_{d}"] = bias.astype(bf)
        a, b_, bias = mk(1, d, 0.5)
        out[f"wih1_{d}"] = a.astype(bf)
        out[f"whh1_{d}"] = b_.astype(bf)
        out[f"b1_{d}"] = bias.astype(bf)
    fcW = np.asarray(inputs["fc_W"], f32)
    fcb = np.asarray(inputs["fc_b"], f32)
    out["wfc"] = _pack_k(0.5 * fcW.T.copy(), 4).astype(bf)
    out["bfc"] = fcb[None, :].astype(bf)
    g = np.asarray(inputs["ln_g"], f32).reshape(3, 128).T.copy()
    bb = np.asarray(inputs["ln_b"], f32).reshape(3, 128).T.copy()
    out["gamma"] = g
    out["beta"] = bb
    out["iota100"] = np.broadcast_to(np.arange(CE, dtype=f32), (128, CE)).copy()
    out["iota15"] = np.broadcast_to(np.arange(K, dtype=f32), (128, K)).copy()
    nl = (np.arange(V) % T != T - 1).astype(f32)
    out["notlast"] = nl.reshape(NCH, 128).T.copy()
    out["id128f"] = np.eye(128, dtype=f32)
    out["id128b"] = np.eye(128, dtype=f32).astype(bf)
    out["id8b"] = np.eye(BL, dtype=f32).astype(bf)
    out["id8f"] = np.eye(BL, dtype=f32)
    tr = np.asarray(inputs["crf_trans"], f32)
    cs = np.asarray(inputs["crf_start"], f32)
    ce_ = np.asarray(inputs["crf_end"], f32)
    out["transf"] = tr
    out["eexp"] = np.exp(tr)
    out["estart"] = np.exp(cs)[:, None]
    out["eend"] = np.exp(ce_)[:, None]
    out["startrep"] = np.broadcast_to(cs, (BL, K)).copy()
    out["endrep"] = np.broadcast_to(ce_, (BL, K)).copy()
    out["ones128f"] = np.ones((128, 1), f32)
    out["ones15f"] = np.ones((K, 1), f32)
    out["ones1x15f"] = np.ones((1, K), f32)
    return out


_NC_CACHE = {}


def kernel(**inputs):
    if "nc" not in _NC_CACHE:
        _NC_CACHE["nc"] = build_nc()
    nc = _NC_CACHE["nc"]
    reps = _prep(inputs)
    emb = np.asarray(inputs["arabert_emb"], np.float32)
    tags = np.asarray(inputs["tags"], np.float32)
    chars = np.asarray(inputs["char_ids"], np.float32)
    in_maps = []
    for c in range(NCORES):
        sl = slice(c * BL, (c + 1) * BL)
        m = dict(reps)
        m["emb"] = emb[sl].reshape(V, D)
        m["tags"] = tags[sl]
        m["chars"] = chars[sl]
        in_maps.append(m)
    bench = bool(int(os.environ.get("KBENCH", "0")))
    res = run_bass_kernel_spmd(nc, in_maps, core_ids=list(range(NCORES)))
    if bench:
        import time as _time
        times = []
        for _ in range(3):
            t0 = _time.perf_counter()
            run_bass_kernel_spmd(nc, in_maps, core_ids=list(range(NCORES)))
            times.append(_time.perf_counter() - t0)
        print(f"HW exec time: {int(min(times) * 1e9)} ns (wall, min of 3)")
    total = np.float32(0.0)
    for r in res.results:
        total += np.float32(r["out"][0, 0])
    return np.asarray(total, np.float32)
